# revision 1
# baseline (speedup 1.0000x reference)
"""HINGCN (metapath GCN) Trainium2 kernel — 8-core SPMD, node-dim sharded.

Reference computation (N=8192, F=128, H=32, M=3 metapaths, C=16 classes):
    h1 = relu(A[m] @ (x @ W1[m]) + b1[m])          per metapath
    h2 = relu(A[m] @ (h1 @ W2[m]) + b2[m])
    e  = leaky_relu(h2 . a, 0.2); attn = softmax_m(e)
    out = sum_m attn[m] * h2[m];  logits = relu(out @ W_lin + b_lin)
    return log_softmax(logits)

Sharding: core k owns output rows u in [1024k, 1024k+1024). Host passes the
transposed row-block AT_k[m] = A[m][rows_k, :].T as bf16 (error impact on the
final output measured at ~1e-5 absolute — far below fp32 reference noise),
x/weights replicated. The [N, H] support matrix for layer 2 is AllGathered
between the layers on-device; everything else is local.

Matmul orientation: h1T[32h, u] = sum_v S1[v-tile][128,32].T @ AT[v-tile][128, u]
so the big A tiles stream through the PE as the moving operand at full rate,
and biases land on the partition axis (plain activation bias). All matmul
operands start at partition 0 (nonzero partition offsets on matmul operands
are broken in this toolchain).
"""

import numpy as np
import ml_dtypes
from contextlib import ExitStack

import concourse.bass as bass
import concourse.tile as tile
from concourse import bacc, mybir
from concourse.bass_utils import run_bass_kernel_spmd
from concourse.masks import make_identity

NCORES = 8
N, F, H, M, C = 8192, 128, 32, 3, 16
UL = N // NCORES          # rows per core (1024)
VT = N // 128             # v-tiles (64)
UT = UL // 128            # u-tiles per core (8)
NSTRIP = UL // 512        # 512-wide psum strips per core (2)
ALPHA = 0.2

BF = mybir.dt.bfloat16
F32 = mybir.dt.float32
AX = mybir.AxisListType.X
AF = mybir.ActivationFunctionType
OP = mybir.AluOpType


def build_kernel_body(nc, tc, ctx, t_in, out_dram):
    xt, at, w1, w2, b1t, b2t, arow, wlin = (
        t_in["xt"], t_in["at"], t_in["w1"], t_in["w2"],
        t_in["b1t"], t_in["b2t"], t_in["arow"], t_in["wlin"])

    const = ctx.enter_context(tc.tile_pool(name="const", bufs=1))
    sbuf = ctx.enter_context(tc.tile_pool(name="sbuf", bufs=2))
    atp = ctx.enter_context(tc.tile_pool(name="atp", bufs=8))
    psum = ctx.enter_context(tc.tile_pool(name="psum", bufs=2, space="PSUM"))
    dram = ctx.enter_context(tc.tile_pool(name="dram", bufs=1, space="DRAM"))

    # ---- constants / parameters in SBUF ----
    xt_sb = const.tile([128, N], BF)
    nc.sync.dma_start(xt_sb[:], xt[:])
    w1_sb = const.tile([128, M * H], BF)
    nc.sync.dma_start(w1_sb[:], w1[:])
    w2_sb = const.tile([H, M * H], BF)
    nc.sync.dma_start(w2_sb[:], w2[:])
    b1t_sb = const.tile([H, M], F32)
    nc.sync.dma_start(b1t_sb[:], b1t[:])
    b2t_sb = const.tile([H, M], F32)
    nc.sync.dma_start(b2t_sb[:], b2t[:])
    arow_sb = const.tile([1, H], F32)
    nc.sync.dma_start(arow_sb[:], arow[:])
    wlin_sb = const.tile([H + 1, C], F32)
    nc.sync.dma_start(wlin_sb[:], wlin[:])
    ones1_sb = const.tile([1, 128], F32)
    nc.vector.memset(ones1_sb[:], 1.0)
    ident_sb = const.tile([128, 128], F32)
    make_identity(nc, ident_sb[:])

    s1_sb = const.tile([128, VT * M * H], BF)       # S1[v, (vt,m,h)]
    s2f_sb = const.tile([128, VT * M * H], BF)      # gathered S2, same layout
    h1t_sb = [const.tile([H, UL], BF, name=f"h1t_{m}") for m in range(M)]
    h2t_sb = [const.tile([H, UL], F32, name=f"h2t_{m}") for m in range(M)]
    abc_sb = const.tile([128, H], F32)              # a broadcast to 128 rows

    # a broadcast via K=1 matmul: [128,1] ones^T . [1,32] a
    psab = psum.tile([128, H], F32, tag="wide")
    nc.tensor.matmul(psab[:], ones1_sb[:], arow_sb[:], start=True, stop=True)
    nc.scalar.copy(abc_sb[:], psab[:])

    # ---- S1 = x @ W1 (all metapaths per matmul) ----
    for vt in range(VT):
        ps1 = psum.tile([128, M * H], F32, tag="wide", name="ps1")
        nc.tensor.matmul(ps1[:], xt_sb[:, vt * 128:(vt + 1) * 128], w1_sb[:],
                         start=True, stop=True)
        nc.vector.tensor_copy(s1_sb[:, vt * M * H:(vt + 1) * M * H], ps1[:])

    # ---- GCN layer: h_t[m] = relu(sum_v S[vt].T @ AT[m, vt] + b) ----
    def gcn_layer(s_sb, bt_sb, ht_out):
        for m in range(M):
            acc = [psum.tile([H, 512], F32, tag="acc", name=f"acc{m}_{s}")
                   for s in range(NSTRIP)]
            for vt in range(VT):
                att = atp.tile([128, UL], BF, tag="at", name="att")
                nc.sync.dma_start(att[:], at[m, vt * 128:(vt + 1) * 128, :])
                lhs = s_sb[:, vt * M * H + m * H: vt * M * H + (m + 1) * H]
                for s in range(NSTRIP):
                    nc.tensor.matmul(acc[s][:], lhs, att[:, s * 512:(s + 1) * 512],
                                     start=(vt == 0), stop=(vt == VT - 1))
            for s in range(NSTRIP):
                nc.scalar.activation(ht_out[m][:, s * 512:(s + 1) * 512], acc[s][:],
                                     AF.Relu, bias=bt_sb[:, m:m + 1])

    gcn_layer(s1_sb, b1t_sb, h1t_sb)

    # ---- S2 = h1 @ W2 -> DRAM -> AllGather ----
    s2loc = dram.tile([M, UL, H], BF)
    for m in range(M):
        for ut in range(UT):
            ps2 = psum.tile([128, H], F32, tag="wide", name="ps2")
            nc.tensor.matmul(ps2[:], h1t_sb[m][:, ut * 128:(ut + 1) * 128],
                             w2_sb[:, m * H:(m + 1) * H], start=True, stop=True)
            st = sbuf.tile([128, H], BF, tag="s2st", name="st")
            nc.vector.tensor_copy(st[:], ps2[:])
            nc.sync.dma_start(s2loc[m, ut * 128:(ut + 1) * 128, :], st[:])

    s2full = dram.tile([NCORES * M, UL, H], BF, addr_space="Shared")
    nc.gpsimd.collective_compute(
        "AllGather", OP.bypass,
        replica_groups=[list(range(NCORES))],
        ins=[s2loc[:].opt()], outs=[s2full[:].opt()])

    # unpack gathered S2 into the same [128, (vt,m,h)] layout as S1
    for r in range(NCORES):
        blk = s2f_sb[:, 8 * r * M * H:(8 * r + 8) * M * H].rearrange(
            "p (ut mh) -> p ut mh", ut=UT)
        for m in range(M):
            # dst: [p, ut, h] with col = (8r+ut)*M*H + m*H + h
            dst = blk[:, :, m * H:(m + 1) * H]
            src = s2full[M * r + m, :, :].rearrange("(ut p) h -> p ut h", p=128)
            nc.sync.dma_start(dst, src)

    gcn_layer(s2f_sb, b2t_sb, h2t_sb)

    # ---- metapath attention + linear head, per 128-row tile ----
    for ut in range(UT):
        h2u = []
        for m in range(M):
            trp = psum.tile([128, H], F32, tag="wide", name="trp")
            nc.tensor.transpose(trp[:], h2t_sb[m][:, ut * 128:(ut + 1) * 128],
                                ident_sb[0:H, 0:H])
            hu = sbuf.tile([128, H], F32, tag=f"h2u{m}", name="hu")
            nc.scalar.copy(hu[:], trp[:])
            h2u.append(hu)
        et = sbuf.tile([128, M], F32, tag="et", name="et")
        for m in range(M):
            tmp = sbuf.tile([128, H], F32, tag="etmp", name="tmp")
            nc.vector.tensor_mul(tmp[:], h2u[m][:], abc_sb[:])
            nc.vector.reduce_sum(et[:, m:m + 1], tmp[:], axis=AX)
        # leaky relu + softmax over metapaths (free dim, M=3)
        eta = sbuf.tile([128, M], F32, tag="eta", name="eta")
        nc.vector.tensor_scalar_mul(eta[:], et[:], ALPHA)
        etl = sbuf.tile([128, M], F32, tag="etl", name="etl")
        nc.vector.tensor_max(etl[:], et[:], eta[:])
        nmx = sbuf.tile([128, 1], F32, tag="nmx", name="nmx")
        nc.vector.reduce_max(nmx[:], etl[:], axis=AX, negate=True)
        ex = sbuf.tile([128, M], F32, tag="ex", name="ex")
        nc.scalar.activation(ex[:], etl[:], AF.Exp, bias=nmx[:])
        ssum = sbuf.tile([128, 1], F32, tag="ssum", name="ssum")
        nc.vector.reduce_sum(ssum[:], ex[:], axis=AX)
        rs = sbuf.tile([128, 1], F32, tag="rs", name="rs")
        nc.vector.reciprocal(rs[:], ssum[:])
        attn = sbuf.tile([128, M], F32, tag="attn", name="attn")
        nc.vector.tensor_scalar_mul(attn[:], ex[:], rs[:])
        # out = sum_m attn[:, m] * h2u[m]
        t0 = sbuf.tile([128, H], F32, tag="t0", name="t0")
        nc.vector.tensor_scalar_mul(t0[:], h2u[0][:], attn[:, 0:1])
        t1 = sbuf.tile([128, H], F32, tag="t1", name="t1")
        nc.vector.tensor_scalar_mul(t1[:], h2u[1][:], attn[:, 1:2])
        t01 = sbuf.tile([128, H], F32, tag="t01", name="t01")
        nc.vector.tensor_add(t01[:], t0[:], t1[:])
        t2 = sbuf.tile([128, H], F32, tag="t2", name="t2")
        nc.vector.tensor_scalar_mul(t2[:], h2u[2][:], attn[:, 2:3])
        oacc = sbuf.tile([128, H], F32, tag="oacc", name="oacc")
        nc.vector.tensor_add(oacc[:], t01[:], t2[:])
        # logits = relu([out, 1] @ [W_lin; b_lin])
        otp = psum.tile([H, 128], F32, tag="tiny", name="otp")
        nc.tensor.transpose(otp[:], oacc[:], ident_sb[:])
        ota = sbuf.tile([H + 1, 128], F32, tag="ota", name="ota")
        nc.scalar.copy(ota[0:H, :], otp[:])
        nc.vector.memset(ota[H:H + 1, :], 1.0)
        lg = psum.tile([128, C], F32, tag="wide", name="lg")
        nc.tensor.matmul(lg[:], ota[:], wlin_sb[:], start=True, stop=True)
        lgr = sbuf.tile([128, C], F32, tag="lgr", name="lgr")
        nc.scalar.activation(lgr[:], lg[:], AF.Relu)
        # log_softmax over classes
        nmx2 = sbuf.tile([128, 1], F32, tag="nmx2", name="nmx2")
        nc.vector.reduce_max(nmx2[:], lgr[:], axis=AX, negate=True)
        ex2 = sbuf.tile([128, C], F32, tag="ex2", name="ex2")
        nc.scalar.activation(ex2[:], lgr[:], AF.Exp, bias=nmx2[:])
        sm = sbuf.tile([128, 1], F32, tag="sm", name="sm")
        nc.vector.reduce_sum(sm[:], ex2[:], axis=AX)
        lssum = sbuf.tile([128, 1], F32, tag="lssum", name="lssum")
        nc.scalar.activation(lssum[:], sm[:], AF.Ln)
        fin = sbuf.tile([128, C], F32, tag="fin", name="fin")
        nc.vector.tensor_scalar(fin[:], lgr[:], nmx2[:], lssum[:],
                                op0=OP.add, op1=OP.subtract)
        nc.sync.dma_start(out_dram[ut * 128:(ut + 1) * 128, :], fin[:])


_CACHED = {}


def build():
    if "nc" in _CACHED:
        return _CACHED["nc"]
    nc = bacc.Bacc("TRN2", target_bir_lowering=False, debug=False,
                   num_devices=NCORES)
    t_in = {
        "xt": nc.dram_tensor("xt", [128, N], BF, kind="ExternalInput").ap(),
        "at": nc.dram_tensor("at", [M, N, UL], BF, kind="ExternalInput").ap(),
        "w1": nc.dram_tensor("w1", [128, M * H], BF, kind="ExternalInput").ap(),
        "w2": nc.dram_tensor("w2", [H, M * H], BF, kind="ExternalInput").ap(),
        "b1t": nc.dram_tensor("b1t", [H, M], F32, kind="ExternalInput").ap(),
        "b2t": nc.dram_tensor("b2t", [H, M], F32, kind="ExternalInput").ap(),
        "arow": nc.dram_tensor("arow", [1, H], F32, kind="ExternalInput").ap(),
        "wlin": nc.dram_tensor("wlin", [H + 1, C], F32, kind="ExternalInput").ap(),
    }
    out_dram = nc.dram_tensor("out", [UL, C], F32, kind="ExternalOutput").ap()
    with tile.TileContext(nc) as tc, ExitStack() as ctx:
        build_kernel_body(nc, tc, ctx, t_in, out_dram)
    nc.compile()
    _CACHED["nc"] = nc
    return nc


def _bf16(x):
    """Fast f32 -> bf16 with round-to-nearest-even via integer ops."""
    x = np.ascontiguousarray(x, dtype=np.float32)
    u = x.view(np.uint32)
    r = ((u + 0x7FFF + ((u >> 16) & 1)) >> 16).astype(np.uint16)
    return r.view(ml_dtypes.bfloat16)


def make_in_maps(x, adjs, W1, b1, W2, b2, a, W_lin, b_lin):
    xt = np.ascontiguousarray(_bf16(x).T)                       # [128, N]
    w1 = np.ascontiguousarray(_bf16(W1).transpose(1, 0, 2)).reshape(128, M * H)
    w2 = np.ascontiguousarray(_bf16(W2).transpose(1, 0, 2)).reshape(H, M * H)
    b1t = np.ascontiguousarray(b1.T, dtype=np.float32)          # [H, M]
    b2t = np.ascontiguousarray(b2.T, dtype=np.float32)
    arow = np.ascontiguousarray(a, dtype=np.float32).reshape(1, H)
    wlin = np.concatenate([W_lin, b_lin[None, :]], axis=0).astype(np.float32)
    adjs_bf = _bf16(adjs)                                       # [M, N, N]
    in_maps = []
    for k in range(NCORES):
        atk = np.ascontiguousarray(
            adjs_bf[:, k * UL:(k + 1) * UL, :].transpose(0, 2, 1))
        in_maps.append({"xt": xt, "at": atk, "w1": w1, "w2": w2,
                        "b1t": b1t, "b2t": b2t, "arow": arow, "wlin": wlin})
    return in_maps


def kernel(x, adjs, W1, b1, W2, b2, a, W_lin, b_lin, _trace=False):
    nc = build()
    in_maps = make_in_maps(x, adjs, W1, b1, W2, b2, a, W_lin, b_lin)
    res = run_bass_kernel_spmd(nc, in_maps, core_ids=list(range(NCORES)),
                               trace=_trace)
    out = np.concatenate([res.results[k]["out"] for k in range(NCORES)], axis=0)
    if _trace:
        kernel.last_result = res
    return out



# revision 8
# speedup vs baseline: 1.6398x; 1.6398x over previous
"""HINGCN (metapath GCN) Trainium2 kernel — 8-core SPMD, node-dim sharded. v2

Reference computation (N=8192, F=128, H=32, M=3 metapaths, C=16 classes):
    h1 = relu(A[m] @ (x @ W1[m]) + b1[m])          per metapath
    h2 = relu(A[m] @ (h1 @ W2[m]) + b2[m])
    e  = leaky_relu(h2 . a, 0.2); attn = softmax_m(e)
    out = sum_m attn[m] * h2[m];  logits = relu(out @ W_lin + b_lin)
    return log_softmax(logits)

Core k owns output rows u in [1024k, 1024k+1024). The big adjacency row-block
is shipped transposed as fp8-e4m3 scaled by N (values land in [0,1); measured
final-output error ~6e-7 relative — the 1/N is folded into W1/W2 so no
on-device rescale is needed). Weights/x replicated; the [N, H] support matrix
is AllGathered between layers.

Performance structure (PE measured ~1.2 GHz on this part, DMA ~358 GB/s/core):
- A streams through the PE as the moving operand; the three metapaths' [128,32]
  stationaries are packed into PE column groups 0/1/2 via PSUM row-slices of a
  shared [128,512] accumulator, so the three matmuls run concurrently.
- A is chunked [128, 8192] (1 MB per DMA, contiguous 8 KB per-partition lines).
  Chunks r<KEEPR stay resident in SBUF and are reused by layer 2, so the
  AllGather latency does not stall the PE and ~9 MB of HBM traffic is saved.
- A-chunk loads ride nc.sync's HWDGE ring exclusively; everything else
  (consts, S2 store/gather-unpack, outputs) rides nc.scalar's ring, so the A
  stream never queues behind a gather-dependent transfer.
- The contraction (v) dimension is processed in a (r, p, l) interleaved order,
  v = 1024 r + 8 p + l, chosen so the gathered S2 blocks [1024, 32] load as
  contiguous [128, 256] SBUF tiles whose column groups directly serve as
  layer-2 stationaries. x's columns are host-permuted to match.
- Tail (attention + head + log_softmax) is batched across all 8 u-tiles:
  2 activation-table loads total, relu via DVE max.
"""

import numpy as np
import ml_dtypes
from contextlib import ExitStack

import concourse.bass as bass
import concourse.tile as tile
from concourse import bacc, mybir
from concourse.bass_utils import run_bass_kernel_spmd
from concourse.masks import make_identity

NCORES = 8
N, F, H, M, C = 8192, 128, 32, 3, 16
UL = N // NCORES          # rows per core (1024)
R = 8                     # v chunk groups (1024 rows each)
L = 8                     # tiles per chunk group
KEEPR = 3                 # chunk groups kept resident across both layers
ALPHA = 0.2

BF = mybir.dt.bfloat16
F8 = mybir.dt.float8e4
F32 = mybir.dt.float32
AX = mybir.AxisListType.X
AF = mybir.ActivationFunctionType
OP = mybir.AluOpType


def build_kernel_body(nc, tc, ctx, t_in, out_dram):
    xtp, at, w1, w2, b1c, b2c, atil, identb, wlin = (
        t_in["xtp"], t_in["at"], t_in["w1"], t_in["w2"],
        t_in["b1c"], t_in["b2c"], t_in["atil"], t_in["identb"], t_in["wlin"])

    const = ctx.enter_context(tc.tile_pool(name="const", bufs=1))
    sbuf = ctx.enter_context(tc.tile_pool(name="sbuf", bufs=2))
    atp = ctx.enter_context(tc.tile_pool(name="atp", bufs=5))
    pacc = ctx.enter_context(tc.tile_pool(name="pacc", bufs=2, space="PSUM"))
    psm = ctx.enter_context(tc.tile_pool(name="psm", bufs=2, space="PSUM"))
    dram = ctx.enter_context(tc.tile_pool(name="dram", bufs=1, space="DRAM"))

    # ---- A-chunk loads: keeps first, then streamed; all on nc.sync ----
    keep = {}
    for r in range(KEEPR):
        for m in range(M):
            t = const.tile([128, L * UL], F8, name=f"keep_{m}_{r}")
            nc.sync.dma_start(t[:], at[m, r, :, :])
            keep[(m, r)] = t

    def chunk_tiles(r):
        if r < KEEPR:
            return [keep[(m, r)] for m in range(M)]
        tiles = []
        for m in range(M):
            t = atp.tile([128, L * UL], F8, tag="at", name=f"at{m}")
            nc.sync.dma_start(t[:], at[m, r, :, :])
            tiles.append(t)
        return tiles

    # ---- constants / parameters (nc.scalar ring) ----
    xtp_sb = const.tile([128, N], BF)
    nc.scalar.dma_start(xtp_sb[:], xtp[:])
    w1_sb = const.tile([128, M * H], BF)
    nc.scalar.dma_start(w1_sb[:], w1[:])
    w2_sb = const.tile([M * H, H], BF)
    nc.scalar.dma_start(w2_sb[:], w2[:])
    b1c_sb = const.tile([M * H, 1], F32)
    nc.scalar.dma_start(b1c_sb[:], b1c[:])
    b2c_sb = const.tile([M * H, 1], F32)
    nc.scalar.dma_start(b2c_sb[:], b2c[:])
    atil_sb = const.tile([128, M * H * 8], F32)
    nc.scalar.dma_start(atil_sb[:], atil[:])
    identb_sb = const.tile([M * H, H], F32)
    nc.scalar.dma_start(identb_sb[:], identb[:])
    wlin_sb = const.tile([H + 1, C], F32)
    nc.scalar.dma_start(wlin_sb[:], wlin[:])
    ident_sb = const.tile([128, 128], F32)
    make_identity(nc, ident_sb[:])

    s1_sb = const.tile([128, R * L * M * H], BF)      # S1[v-tile t, (m,h)]
    s2r_sb = const.tile([128, R * M * L * H], BF)     # gathered S2, tile (r,m)
    h1t_sb = const.tile([M * H, UL], BF)              # rows 32m+h
    h2t_sb = const.tile([M * H, UL], F32)

    # ---- S1 = x @ (W1/N), v-tile at a time ----
    for t in range(R * L):
        ps1 = psm.tile([128, M * H], F32, tag="w96", name="ps1")
        nc.tensor.matmul(ps1[:], xtp_sb[:, t * 128:(t + 1) * 128], w1_sb[:],
                         start=True, stop=True)
        nc.vector.tensor_copy(s1_sb[:, t * M * H:(t + 1) * M * H], ps1[:])

    # ---- GCN layer: three metapaths concurrent in PE column groups ----
    def gcn_layer(stat, bc_sb, ht_out):
        accs = [pacc.tile([128, 512], F32, tag="acc", name=f"acc{s}")
                for s in range(2)]
        for r in range(R):
            ats = chunk_tiles(r)
            for l in range(L):
                t = L * r + l
                for s in range(2):
                    for m in range(M):
                        nc.tensor.matmul(
                            accs[s][32 * m:32 * m + 32, :], stat(t, m),
                            ats[m][:, l * UL + s * 512: l * UL + s * 512 + 512],
                            start=(t == 0), stop=(t == R * L - 1),
                            skip_group_check=True)
        for s in range(2):
            for m in range(M):
                nc.scalar.activation(
                    ht_out[32 * m:32 * m + 32, s * 512:(s + 1) * 512],
                    accs[s][32 * m:32 * m + 32, :], AF.Relu,
                    bias=bc_sb[32 * m:32 * m + 32, :])

    gcn_layer(lambda t, m: s1_sb[:, t * M * H + m * H: t * M * H + (m + 1) * H],
              b1c_sb, h1t_sb)

    # ---- S2 = h1 @ (W2/N) -> DRAM -> AllGather ----
    s2loc = dram.tile([M, UL, H], BF)
    for ut in range(L):
        for m in range(M):
            ps2 = psm.tile([128, H], F32, tag="w32", name="ps2")
            nc.tensor.matmul(ps2[:], h1t_sb[32 * m:32 * m + 32, ut * 128:(ut + 1) * 128],
                             w2_sb[32 * m:32 * m + 32, :], start=True, stop=True)
            st = sbuf.tile([128, H], BF, tag="s2st", name="st")
            nc.vector.tensor_copy(st[:], ps2[:])
            nc.scalar.dma_start(s2loc[m, ut * 128:(ut + 1) * 128, :], st[:])

    s2full = dram.tile([NCORES * M, UL, H], BF, addr_space="Shared")
    nc.gpsimd.collective_compute(
        "AllGather", OP.bypass,
        replica_groups=[list(range(NCORES))],
        ins=[s2loc[:].opt()], outs=[s2full[:].opt()])

    # gathered block (r,m) = [1024, 32] loads as contiguous [128, 256]:
    # partition p holds rows 8p..8p+7, so column group l is the (r,l) v-tile.
    for r in range(NCORES):
        for m in range(M):
            src = s2full[r * M + m, :, :].rearrange("(p q) h -> p (q h)", p=128)
            nc.scalar.dma_start(
                s2r_sb[:, (r * M + m) * 256:(r * M + m + 1) * 256], src)

    gcn_layer(lambda t, m: s2r_sb[:, ((t // L) * M + m) * 256 + (t % L) * H:
                                  ((t // L) * M + m) * 256 + (t % L + 1) * H],
              b2c_sb, h2t_sb)

    # ---- tail: metapath attention + linear head, batched over u-tiles ----
    h2u = const.tile([128, L * M * H], F32)       # [u, (ut, m, h)]
    for ut in range(L):
        for m in range(M):
            trp = psm.tile([128, H], F32, tag="w32", name="trp")
            nc.tensor.transpose(trp[:], h2t_sb[32 * m:32 * m + 32,
                                               ut * 128:(ut + 1) * 128],
                                identb_sb[32 * m:32 * m + 32, :])
            nc.vector.tensor_copy(
                h2u[:, (ut * M + m) * H:(ut * M + m + 1) * H], trp[:])

    tmp = sbuf.tile([128, L * M * H], F32, tag="tmp", name="tmp")
    nc.vector.tensor_mul(tmp[:], h2u[:], atil_sb[:])
    e_all = sbuf.tile([128, L * M], F32, tag="eall", name="e_all")
    nc.vector.reduce_sum(e_all[:].rearrange("p (a o) -> p a o", o=1),
                         tmp[:].rearrange("p (a h) -> p a h", h=H), axis=AX)
    # leaky relu then unshifted softmax over m (e is tiny; exp is safe)
    eta = sbuf.tile([128, L * M], F32, tag="eta", name="eta")
    nc.vector.tensor_scalar_mul(eta[:], e_all[:], ALPHA)
    el = sbuf.tile([128, L * M], F32, tag="el", name="el")
    nc.vector.tensor_max(el[:], e_all[:], eta[:])
    ex = sbuf.tile([128, L * M], F32, tag="ex", name="ex")
    nc.scalar.activation(ex[:], el[:], AF.Exp)
    ssum = sbuf.tile([128, L], F32, tag="ssum", name="ssum")
    nc.vector.reduce_sum(ssum[:].rearrange("p (a o) -> p a o", o=1),
                         ex[:].rearrange("p (a m) -> p a m", m=M), axis=AX)
    rs = sbuf.tile([128, L], F32, tag="rs", name="rs")
    nc.vector.reciprocal(rs[:], ssum[:])
    rs3 = sbuf.tile([128, L * M], F32, tag="rs3", name="rs3")
    for m in range(M):
        nc.vector.tensor_copy(
            rs3[:].rearrange("p (a m) -> p a m", m=M)[:, :, m:m + 1],
            rs[:].rearrange("p (a o) -> p a o", o=1))
    attn = sbuf.tile([128, L * M], F32, tag="attn", name="attn")
    nc.vector.tensor_mul(attn[:], ex[:], rs3[:])
    # out = sum_m attn[:, (ut,m)] * h2u[:, (ut,m,:)]
    oacc = sbuf.tile([128, L * H], F32, tag="oacc", name="oacc")
    t0 = sbuf.tile([128, H], F32, tag="t0", name="t0")
    for ut in range(L):
        nc.vector.tensor_scalar_mul(oacc[:, ut * H:(ut + 1) * H],
                                    h2u[:, (ut * M) * H:(ut * M + 1) * H],
                                    attn[:, ut * M:ut * M + 1])
        for m in (1, 2):
            nc.vector.tensor_scalar_mul(
                t0[:], h2u[:, (ut * M + m) * H:(ut * M + m + 1) * H],
                attn[:, ut * M + m:ut * M + m + 1])
            nc.vector.tensor_add(oacc[:, ut * H:(ut + 1) * H],
                                 oacc[:, ut * H:(ut + 1) * H], t0[:])
    # logits = relu([out, 1] @ [W_lin; b_lin]); log_softmax (unshifted, small)
    ota = const.tile([H + 1, L * 128], F32)
    nc.vector.memset(ota[H:H + 1, :], 1.0)
    for ut in range(L):
        otp = psm.tile([H, 128], F32, tag="w32t", name="otp")
        nc.tensor.transpose(otp[:], oacc[:, ut * H:(ut + 1) * H], ident_sb[:])
        nc.vector.tensor_copy(ota[0:H, ut * 128:(ut + 1) * 128], otp[:])
    lgr = sbuf.tile([128, L * C], F32, tag="lgr", name="lgr")
    for ut in range(L):
        lg = psm.tile([128, C], F32, tag="w32", name="lg")
        nc.tensor.matmul(lg[:], ota[:, ut * 128:(ut + 1) * 128], wlin_sb[:],
                         start=True, stop=True)
        nc.vector.tensor_copy(lgr[:, ut * C:(ut + 1) * C], lg[:])
    lgR = sbuf.tile([128, L * C], F32, tag="lgR", name="lgR")
    nc.vector.tensor_scalar_max(lgR[:], lgr[:], 0.0)
    ex2 = sbuf.tile([128, L * C], F32, tag="ex2", name="ex2")
    nc.scalar.activation(ex2[:], lgR[:], AF.Exp)
    sm = sbuf.tile([128, L], F32, tag="sm", name="sm")
    nc.vector.reduce_sum(sm[:].rearrange("p (a o) -> p a o", o=1),
                         ex2[:].rearrange("p (a c) -> p a c", c=C), axis=AX)
    lssum = sbuf.tile([128, L], F32, tag="lssum", name="lssum")
    nc.scalar.activation(lssum[:], sm[:], AF.Ln)
    fin = sbuf.tile([128, L * C], F32, tag="fin", name="fin")
    for ut in range(L):
        nc.vector.tensor_scalar_sub(fin[:, ut * C:(ut + 1) * C],
                                    lgR[:, ut * C:(ut + 1) * C],
                                    lssum[:, ut:ut + 1])
        nc.scalar.dma_start(out_dram[ut * 128:(ut + 1) * 128, :],
                            fin[:, ut * C:(ut + 1) * C])


_CACHED = {}


def build():
    if "nc" in _CACHED:
        return _CACHED["nc"]
    nc = bacc.Bacc("TRN2", target_bir_lowering=False, debug=False,
                   num_devices=NCORES)
    t_in = {
        "xtp": nc.dram_tensor("xtp", [128, N], BF, kind="ExternalInput").ap(),
        "at": nc.dram_tensor("at", [M, R, 128, L * UL], F8,
                             kind="ExternalInput").ap(),
        "w1": nc.dram_tensor("w1", [128, M * H], BF, kind="ExternalInput").ap(),
        "w2": nc.dram_tensor("w2", [M * H, H], BF, kind="ExternalInput").ap(),
        "b1c": nc.dram_tensor("b1c", [M * H, 1], F32, kind="ExternalInput").ap(),
        "b2c": nc.dram_tensor("b2c", [M * H, 1], F32, kind="ExternalInput").ap(),
        "atil": nc.dram_tensor("atil", [128, M * H * 8], F32,
                               kind="ExternalInput").ap(),
        "identb": nc.dram_tensor("identb", [M * H, H], F32,
                                 kind="ExternalInput").ap(),
        "wlin": nc.dram_tensor("wlin", [H + 1, C], F32,
                               kind="ExternalInput").ap(),
    }
    out_dram = nc.dram_tensor("out", [UL, C], F32, kind="ExternalOutput").ap()
    with tile.TileContext(nc) as tc, ExitStack() as ctx:
        build_kernel_body(nc, tc, ctx, t_in, out_dram)
    nc.compile()
    _CACHED["nc"] = nc
    return nc


def _bf16(x):
    """Fast f32 -> bf16 with round-to-nearest-even via integer ops."""
    x = np.ascontiguousarray(x, dtype=np.float32)
    u = x.view(np.uint32)
    r = ((u + 0x7FFF + ((u >> 16) & 1)) >> 16).astype(np.uint16)
    return r.view(ml_dtypes.bfloat16)


def make_in_maps(x, adjs, W1, b1, W2, b2, a, W_lin, b_lin):
    # x columns permuted to the (r, l, j) v-order: v = 1024 r + 8 j + l
    xt = np.asarray(_bf16(x).T)                                  # [128, N]
    xtp = np.ascontiguousarray(
        xt.reshape(128, R, 128, L).transpose(0, 1, 3, 2)).reshape(128, N)
    w1 = np.ascontiguousarray(
        _bf16(np.asarray(W1) * (1.0 / N)).transpose(1, 0, 2)).reshape(128, M * H)
    w2 = np.ascontiguousarray(
        _bf16(np.asarray(W2) * (1.0 / N))).reshape(M * H, H)
    b1c = np.ascontiguousarray(b1, dtype=np.float32).reshape(M * H, 1)
    b2c = np.ascontiguousarray(b2, dtype=np.float32).reshape(M * H, 1)
    atil = np.tile(np.asarray(a, dtype=np.float32), (128, L * M))
    identb = np.tile(np.eye(H, dtype=np.float32), (M, 1))
    wlin = np.concatenate([W_lin, b_lin[None, :]], axis=0).astype(np.float32)
    adjs8 = (np.asarray(adjs, dtype=np.float32) * float(N)).astype(
        ml_dtypes.float8_e4m3)                                   # [M, N, N]
    in_maps = []
    for k in range(NCORES):
        atk = np.ascontiguousarray(
            adjs8[:, k * UL:(k + 1) * UL, :].transpose(0, 2, 1)
        ).reshape(M, R, 128, L * UL)
        in_maps.append({"xtp": xtp, "at": atk, "w1": w1, "w2": w2,
                        "b1c": b1c, "b2c": b2c, "atil": atil,
                        "identb": identb, "wlin": wlin})
    return in_maps


def kernel(x, adjs, W1, b1, W2, b2, a, W_lin, b_lin, _trace=False):
    nc = build()
    in_maps = make_in_maps(x, adjs, W1, b1, W2, b2, a, W_lin, b_lin)
    res = run_bass_kernel_spmd(nc, in_maps, core_ids=list(range(NCORES)),
                               trace=_trace)
    out = np.concatenate([res.results[k]["out"] for k in range(NCORES)], axis=0)
    if _trace:
        kernel.last_result = res
    return out


# revision 17
# speedup vs baseline: 1.8819x; 1.1476x over previous
"""HINGCN (metapath GCN) Trainium2 kernel — 8-core SPMD, node-dim sharded. v2

Reference computation (N=8192, F=128, H=32, M=3 metapaths, C=16 classes):
    h1 = relu(A[m] @ (x @ W1[m]) + b1[m])          per metapath
    h2 = relu(A[m] @ (h1 @ W2[m]) + b2[m])
    e  = leaky_relu(h2 . a, 0.2); attn = softmax_m(e)
    out = sum_m attn[m] * h2[m];  logits = relu(out @ W_lin + b_lin)
    return log_softmax(logits)

Core k owns output rows u in [1024k, 1024k+1024). The big adjacency row-block
is shipped transposed as fp8-e4m3 scaled by N (values land in [0,1); measured
final-output error ~6e-7 relative — the 1/N is folded into W1/W2 so no
on-device rescale is needed). Weights/x replicated; the [N, H] support matrix
is AllGathered between layers.

Performance structure (PE measured ~1.2 GHz on this part, DMA ~358 GB/s/core):
- A streams through the PE as the moving operand; the three metapaths' [128,32]
  stationaries are packed into PE column groups 0/1/2 via PSUM row-slices of a
  shared [128,512] accumulator, so the three matmuls run concurrently.
- A is chunked [128, 8192] (1 MB per DMA, contiguous 8 KB per-partition lines).
  Chunks r<KEEPR stay resident in SBUF and are reused by layer 2, so the
  AllGather latency does not stall the PE and ~9 MB of HBM traffic is saved.
- A-chunk loads ride nc.sync's HWDGE ring exclusively; everything else
  (consts, S2 store/gather-unpack, outputs) rides nc.scalar's ring, so the A
  stream never queues behind a gather-dependent transfer. Keep and stream
  chunk issues are interleaved so the DMA engines never go idle early.
- The contraction (v) tile (r, l) covers rows v = 1024 r + 128 l + p (plain
  order; the host transposes chunk-internally to [p, l, u]). S2 results are
  copied straight into a [128, M*256] send tile in the slot layout, stored
  with ONE DMA, AllGathered, and re-loaded with ONE DMA — slot column groups
  directly serve as layer-2 stationaries.
- Tail (attention + head + log_softmax) is batched across all 8 u-tiles:
  2 activation-table loads total, relu via DVE max.
"""

import numpy as np
import ml_dtypes
from contextlib import ExitStack

import concourse.bass as bass
import concourse.tile as tile
from concourse import bacc, mybir
from concourse.bass_utils import run_bass_kernel_spmd
from concourse.masks import make_identity

NCORES = 8
N, F, H, M, C = 8192, 128, 32, 3, 16
UL = N // NCORES          # rows per core (1024)
R = 8                     # v chunk groups (1024 rows each)
L = 8                     # tiles per chunk group
KEEPR = 4                 # chunk groups kept resident across both layers
ALPHA = 0.2

BF = mybir.dt.bfloat16
F8 = mybir.dt.float8e4
F32 = mybir.dt.float32
AX = mybir.AxisListType.X
AF = mybir.ActivationFunctionType
OP = mybir.AluOpType


def build_kernel_body(nc, tc, ctx, t_in, out_dram):
    xtp, at, w1, w2, b1c, b2c, atil, identb, wlin = (
        t_in["xtp"], t_in["at"], t_in["w1"], t_in["w2"],
        t_in["b1c"], t_in["b2c"], t_in["atil"], t_in["identb"], t_in["wlin"])

    const = ctx.enter_context(tc.tile_pool(name="const", bufs=1))
    sbuf = ctx.enter_context(tc.tile_pool(name="sbuf", bufs=2))
    atp = ctx.enter_context(tc.tile_pool(name="atp", bufs=4))
    pacc = ctx.enter_context(tc.tile_pool(name="pacc", bufs=2, space="PSUM"))
    psm = ctx.enter_context(tc.tile_pool(name="psm", bufs=2, space="PSUM"))
    dram = ctx.enter_context(tc.tile_pool(name="dram", bufs=1, space="DRAM"))

    # ---- A-chunk loads on nc.sync: keep/stream issue interleaved so the
    # DMA engines run flat out from t=0 while the PE works the early keeps.
    keep = {}
    l1_stream = {}

    def issue_chunk(r, store):
        for m in range(M):
            if store is keep:
                t = const.tile([128, L * UL], F8, name=f"keep_{m}_{r}")
            else:
                t = atp.tile([128, L * UL], F8, tag="at", name=f"at{m}")
            nc.sync.dma_start(t[:], at[m, r, :, :])
            store[(m, r)] = t

    issue_order = []
    srs = list(range(KEEPR, R))
    for i in range(KEEPR):
        issue_order.append((i, True))
        if i < len(srs):
            issue_order.append((srs[i], False))
    issue_order += [(r, False) for r in srs[KEEPR:]]
    for r, is_keep in issue_order:
        issue_chunk(r, keep if is_keep else l1_stream)

    def chunk_tiles(r, stream):
        if r < KEEPR:
            return [keep[(m, r)] for m in range(M)]
        return [stream[(m, r)] for m in range(M)]

    # ---- constants / parameters (nc.scalar ring) ----
    xtp_sb = const.tile([128, N], BF)
    nc.scalar.dma_start(xtp_sb[:], xtp[:])
    w1_sb = const.tile([128, M * H], BF)
    nc.scalar.dma_start(w1_sb[:], w1[:])
    w2_sb = const.tile([M * H, H], BF)
    nc.scalar.dma_start(w2_sb[:], w2[:])
    b1c_sb = const.tile([M * H, 1], F32)
    nc.scalar.dma_start(b1c_sb[:], b1c[:])
    b2c_sb = const.tile([M * H, 1], F32)
    nc.scalar.dma_start(b2c_sb[:], b2c[:])
    atil_sb = const.tile([128, M * H * 8], F32)
    nc.scalar.dma_start(atil_sb[:], atil[:])
    identb_sb = const.tile([M * H, H], F32)
    nc.scalar.dma_start(identb_sb[:], identb[:])
    wlin_sb = const.tile([H + 1, C], F32)
    nc.scalar.dma_start(wlin_sb[:], wlin[:])
    ident_sb = const.tile([128, 128], F32)
    make_identity(nc, ident_sb[:])

    s1_sb = const.tile([128, R * L * M * H], BF)      # S1[v-tile t, (m,h)]
    s2r_sb = const.tile([128, R * M * L * H], BF)     # gathered S2, tile (r,m)
    h1t_sb = const.tile([M * H, UL], BF)              # rows 32m+h
    h2t_sb = const.tile([M * H, UL], F32)

    # ---- S1 = x @ (W1/N), v-tile at a time ----
    for t in range(R * L):
        ps1 = psm.tile([128, M * H], F32, tag="w96", name="ps1")
        nc.tensor.matmul(ps1[:], xtp_sb[:, t * 128:(t + 1) * 128], w1_sb[:],
                         start=True, stop=True)
        nc.vector.tensor_copy(s1_sb[:, t * M * H:(t + 1) * M * H], ps1[:])

    # ---- GCN layer: three metapaths concurrent in PE column groups ----
    def gcn_layer(stat, bc_sb, ht_out, stream):
        accs = [pacc.tile([128, 512], F32, tag="acc", name=f"acc{s}")
                for s in range(2)]
        for r in range(R):
            ats = chunk_tiles(r, stream)
            for l in range(L):
                t = L * r + l
                for s in range(2):
                    for m in range(M):
                        nc.tensor.matmul(
                            accs[s][32 * m:32 * m + 32, :], stat(t, m),
                            ats[m][:, l * UL + s * 512: l * UL + s * 512 + 512],
                            start=(t == 0), stop=(t == R * L - 1),
                            skip_group_check=True)
        for s in range(2):
            for m in range(M):
                nc.scalar.activation(
                    ht_out[32 * m:32 * m + 32, s * 512:(s + 1) * 512],
                    accs[s][32 * m:32 * m + 32, :], AF.Relu,
                    bias=bc_sb[32 * m:32 * m + 32, :])

    gcn_layer(lambda t, m: s1_sb[:, t * M * H + m * H: t * M * H + (m + 1) * H],
              b1c_sb, h1t_sb, l1_stream)

    # layer-2 re-streams of the non-kept chunks: issued on the sync ring right
    # after layer 1's, so they fill the pool while the AllGather runs.
    l2_stream = {}
    for r in range(KEEPR, R):
        issue_chunk(r, l2_stream)

    # ---- S2 = h1 @ (W2/N), copied straight into the slot layout ----
    # slot column group l of metapath block m holds rows u = 128 l + p, which
    # is exactly the ps2 tile of u-tile l.
    s2send = const.tile([128, M * L * H], BF)
    for ut in range(L):
        for m in range(M):
            ps2 = psm.tile([128, H], F32, tag="w32", name="ps2")
            nc.tensor.matmul(ps2[:], h1t_sb[32 * m:32 * m + 32, ut * 128:(ut + 1) * 128],
                             w2_sb[32 * m:32 * m + 32, :], start=True, stop=True)
            nc.vector.tensor_copy(
                s2send[:, m * L * H + ut * H:m * L * H + (ut + 1) * H], ps2[:])

    s2loc = dram.tile([128, M * L * H], BF)
    nc.scalar.dma_start(s2loc[:, :], s2send[:])
    s2full = dram.tile([NCORES, 128, M * L * H], BF, addr_space="Shared")
    nc.gpsimd.collective_compute(
        "AllGather", OP.bypass,
        replica_groups=[list(range(NCORES))],
        ins=[s2loc[:].opt()], outs=[s2full[:].opt()])
    nc.scalar.dma_start(s2r_sb[:].rearrange("p (r c) -> p r c", r=NCORES),
                        s2full[:, :, :].rearrange("r p c -> p r c"))

    gcn_layer(lambda t, m: s2r_sb[:, (t // L) * M * L * H + m * L * H + (t % L) * H:
                                  (t // L) * M * L * H + m * L * H + (t % L + 1) * H],
              b2c_sb, h2t_sb, l2_stream)

    # ---- tail: metapath attention + linear head, batched over u-tiles ----
    h2u = const.tile([128, L * M * H], F32)       # [u, (ut, m, h)]
    for ut in range(L):
        for m in range(M):
            trp = psm.tile([128, H], F32, tag="w32", name="trp")
            nc.tensor.transpose(trp[:], h2t_sb[32 * m:32 * m + 32,
                                               ut * 128:(ut + 1) * 128],
                                identb_sb[32 * m:32 * m + 32, :])
            nc.vector.tensor_copy(
                h2u[:, (ut * M + m) * H:(ut * M + m + 1) * H], trp[:])

    tmp = sbuf.tile([128, L * M * H], F32, tag="tmp", name="tmp")
    nc.vector.tensor_mul(tmp[:], h2u[:], atil_sb[:])
    e_all = sbuf.tile([128, L * M], F32, tag="eall", name="e_all")
    nc.vector.reduce_sum(e_all[:].rearrange("p (a o) -> p a o", o=1),
                         tmp[:].rearrange("p (a h) -> p a h", h=H), axis=AX)
    # leaky relu then unshifted softmax over m (e is tiny; exp is safe)
    eta = sbuf.tile([128, L * M], F32, tag="eta", name="eta")
    nc.vector.tensor_scalar_mul(eta[:], e_all[:], ALPHA)
    el = sbuf.tile([128, L * M], F32, tag="el", name="el")
    nc.vector.tensor_max(el[:], e_all[:], eta[:])
    ex = sbuf.tile([128, L * M], F32, tag="ex", name="ex")
    nc.scalar.activation(ex[:], el[:], AF.Exp)
    ssum = sbuf.tile([128, L], F32, tag="ssum", name="ssum")
    nc.vector.reduce_sum(ssum[:].rearrange("p (a o) -> p a o", o=1),
                         ex[:].rearrange("p (a m) -> p a m", m=M), axis=AX)
    rs = sbuf.tile([128, L], F32, tag="rs", name="rs")
    nc.vector.reciprocal(rs[:], ssum[:])
    rs3 = sbuf.tile([128, L * M], F32, tag="rs3", name="rs3")
    for m in range(M):
        nc.vector.tensor_copy(
            rs3[:].rearrange("p (a m) -> p a m", m=M)[:, :, m:m + 1],
            rs[:].rearrange("p (a o) -> p a o", o=1))
    attn = sbuf.tile([128, L * M], F32, tag="attn", name="attn")
    nc.vector.tensor_mul(attn[:], ex[:], rs3[:])
    # out = sum_m attn[:, (ut,m)] * h2u[:, (ut,m,:)]
    oacc = sbuf.tile([128, L * H], F32, tag="oacc", name="oacc")
    t0 = sbuf.tile([128, H], F32, tag="t0", name="t0")
    for ut in range(L):
        nc.vector.tensor_scalar_mul(oacc[:, ut * H:(ut + 1) * H],
                                    h2u[:, (ut * M) * H:(ut * M + 1) * H],
                                    attn[:, ut * M:ut * M + 1])
        for m in (1, 2):
            nc.vector.tensor_scalar_mul(
                t0[:], h2u[:, (ut * M + m) * H:(ut * M + m + 1) * H],
                attn[:, ut * M + m:ut * M + m + 1])
            nc.vector.tensor_add(oacc[:, ut * H:(ut + 1) * H],
                                 oacc[:, ut * H:(ut + 1) * H], t0[:])
    # logits = relu([out, 1] @ [W_lin; b_lin]); log_softmax (unshifted, small)
    ota = const.tile([H + 1, L * 128], F32)
    nc.vector.memset(ota[H:H + 1, :], 1.0)
    for ut in range(L):
        otp = psm.tile([H, 128], F32, tag="w32t", name="otp")
        nc.tensor.transpose(otp[:], oacc[:, ut * H:(ut + 1) * H], ident_sb[:])
        nc.vector.tensor_copy(ota[0:H, ut * 128:(ut + 1) * 128], otp[:])
    lgr = sbuf.tile([128, L * C], F32, tag="lgr", name="lgr")
    for ut in range(L):
        lg = psm.tile([128, C], F32, tag="w32", name="lg")
        nc.tensor.matmul(lg[:], ota[:, ut * 128:(ut + 1) * 128], wlin_sb[:],
                         start=True, stop=True)
        nc.vector.tensor_copy(lgr[:, ut * C:(ut + 1) * C], lg[:])
    lgR = sbuf.tile([128, L * C], F32, tag="lgR", name="lgR")
    nc.vector.tensor_scalar_max(lgR[:], lgr[:], 0.0)
    ex2 = sbuf.tile([128, L * C], F32, tag="ex2", name="ex2")
    nc.scalar.activation(ex2[:], lgR[:], AF.Exp)
    sm = sbuf.tile([128, L], F32, tag="sm", name="sm")
    nc.vector.reduce_sum(sm[:].rearrange("p (a o) -> p a o", o=1),
                         ex2[:].rearrange("p (a c) -> p a c", c=C), axis=AX)
    lssum = sbuf.tile([128, L], F32, tag="lssum", name="lssum")
    nc.scalar.activation(lssum[:], sm[:], AF.Ln)
    fin = sbuf.tile([128, L * C], F32, tag="fin", name="fin")
    for ut in range(L):
        nc.vector.tensor_scalar_sub(fin[:, ut * C:(ut + 1) * C],
                                    lgR[:, ut * C:(ut + 1) * C],
                                    lssum[:, ut:ut + 1])
        nc.scalar.dma_start(out_dram[ut * 128:(ut + 1) * 128, :],
                            fin[:, ut * C:(ut + 1) * C])


_CACHED = {}


def build():
    if "nc" in _CACHED:
        return _CACHED["nc"]
    nc = bacc.Bacc("TRN2", target_bir_lowering=False, debug=False,
                   num_devices=NCORES)
    t_in = {
        "xtp": nc.dram_tensor("xtp", [128, N], BF, kind="ExternalInput").ap(),
        "at": nc.dram_tensor("at", [M, R, 128, L * UL], F8,
                             kind="ExternalInput").ap(),
        "w1": nc.dram_tensor("w1", [128, M * H], BF, kind="ExternalInput").ap(),
        "w2": nc.dram_tensor("w2", [M * H, H], BF, kind="ExternalInput").ap(),
        "b1c": nc.dram_tensor("b1c", [M * H, 1], F32, kind="ExternalInput").ap(),
        "b2c": nc.dram_tensor("b2c", [M * H, 1], F32, kind="ExternalInput").ap(),
        "atil": nc.dram_tensor("atil", [128, M * H * 8], F32,
                               kind="ExternalInput").ap(),
        "identb": nc.dram_tensor("identb", [M * H, H], F32,
                                 kind="ExternalInput").ap(),
        "wlin": nc.dram_tensor("wlin", [H + 1, C], F32,
                               kind="ExternalInput").ap(),
    }
    out_dram = nc.dram_tensor("out", [UL, C], F32, kind="ExternalOutput").ap()
    with tile.TileContext(nc) as tc, ExitStack() as ctx:
        build_kernel_body(nc, tc, ctx, t_in, out_dram)
    nc.compile()
    _CACHED["nc"] = nc
    return nc


def _bf16(x):
    """Fast f32 -> bf16 with round-to-nearest-even via integer ops."""
    x = np.ascontiguousarray(x, dtype=np.float32)
    u = x.view(np.uint32)
    r = ((u + 0x7FFF + ((u >> 16) & 1)) >> 16).astype(np.uint16)
    return r.view(ml_dtypes.bfloat16)


def make_in_maps(x, adjs, W1, b1, W2, b2, a, W_lin, b_lin):
    xtp = np.ascontiguousarray(_bf16(x).T)                       # [128, N]
    w1 = np.ascontiguousarray(
        _bf16(np.asarray(W1) * (1.0 / N)).transpose(1, 0, 2)).reshape(128, M * H)
    w2 = np.ascontiguousarray(
        _bf16(np.asarray(W2) * (1.0 / N))).reshape(M * H, H)
    b1c = np.ascontiguousarray(b1, dtype=np.float32).reshape(M * H, 1)
    b2c = np.ascontiguousarray(b2, dtype=np.float32).reshape(M * H, 1)
    atil = np.tile(np.asarray(a, dtype=np.float32), (128, L * M))
    identb = np.tile(np.eye(H, dtype=np.float32), (M, 1))
    wlin = np.concatenate([W_lin, b_lin[None, :]], axis=0).astype(np.float32)
    adjs8 = (np.asarray(adjs, dtype=np.float32) * float(N)).astype(
        ml_dtypes.float8_e4m3)                                   # [M, N, N]
    in_maps = []
    for k in range(NCORES):
        # chunk r: [128 p, (l, u)] with v = 1024 r + 128 l + p
        atk = np.ascontiguousarray(
            adjs8[:, k * UL:(k + 1) * UL, :].transpose(0, 2, 1)
            .reshape(M, R, L, 128, UL).transpose(0, 1, 3, 2, 4)
        ).reshape(M, R, 128, L * UL)
        in_maps.append({"xtp": xtp, "at": atk, "w1": w1, "w2": w2,
                        "b1c": b1c, "b2c": b2c, "atil": atil,
                        "identb": identb, "wlin": wlin})
    return in_maps


def kernel(x, adjs, W1, b1, W2, b2, a, W_lin, b_lin, _trace=False):
    nc = build()
    in_maps = make_in_maps(x, adjs, W1, b1, W2, b2, a, W_lin, b_lin)
    res = run_bass_kernel_spmd(nc, in_maps, core_ids=list(range(NCORES)),
                               trace=_trace)
    out = np.concatenate([res.results[k]["out"] for k in range(NCORES)], axis=0)
    if _trace:
        kernel.last_result = res
    return out


# revision 25
# speedup vs baseline: 1.9029x; 1.0111x over previous
"""HINGCN (metapath GCN) Trainium2 kernel — 8-core SPMD, node-dim sharded. v2

Reference computation (N=8192, F=128, H=32, M=3 metapaths, C=16 classes):
    h1 = relu(A[m] @ (x @ W1[m]) + b1[m])          per metapath
    h2 = relu(A[m] @ (h1 @ W2[m]) + b2[m])
    e  = leaky_relu(h2 . a, 0.2); attn = softmax_m(e)
    out = sum_m attn[m] * h2[m];  logits = relu(out @ W_lin + b_lin)
    return log_softmax(logits)

Core k owns output rows u in [1024k, 1024k+1024). The big adjacency row-block
is shipped transposed as fp8-e4m3 scaled by N (values land in [0,1); measured
final-output error ~6e-7 relative — the 1/N is folded into W1/W2 so no
on-device rescale is needed). Weights/x replicated; the [N, H] support matrix
is AllGathered between layers.

Performance structure (PE measured ~1.2 GHz on this part, DMA ~358 GB/s/core):
- A streams through the PE as the moving operand; the three metapaths' [128,32]
  stationaries are packed into PE column groups 0/1/2 via PSUM row-slices of a
  shared [128,512] accumulator, so the three matmuls run concurrently.
- A is chunked [128, 8192] (1 MB per DMA, contiguous 8 KB per-partition lines).
  Chunks r<KEEPR stay resident in SBUF and are reused by layer 2, so the
  AllGather latency does not stall the PE and ~9 MB of HBM traffic is saved.
- A-chunk loads ride nc.sync's HWDGE ring exclusively; everything else
  (consts, S2 store/gather-unpack, outputs) rides nc.scalar's ring, so the A
  stream never queues behind a gather-dependent transfer. Keep and stream
  chunk issues are interleaved so the DMA engines never go idle early.
- The contraction (v) tile (r, l) covers rows v = 1024 r + 128 l + p (plain
  order; the host transposes chunk-internally to [p, l, u]). S2 results are
  copied straight into a [128, M*256] send tile in the slot layout, stored
  with ONE DMA, AllGathered, and re-loaded with ONE DMA — slot column groups
  directly serve as layer-2 stationaries.
- Tail (attention + head + log_softmax) is batched across all 8 u-tiles:
  2 activation-table loads total, relu via DVE max.
"""

import numpy as np
import ml_dtypes
from contextlib import ExitStack

import concourse.bass as bass
import concourse.tile as tile
from concourse import bacc, mybir
from concourse.bass_utils import run_bass_kernel_spmd
from concourse.masks import make_identity

NCORES = 8
N, F, H, M, C = 8192, 128, 32, 3, 16
UL = N // NCORES          # rows per core (1024)
R = 8                     # v chunk groups (1024 rows each)
L = 8                     # tiles per chunk group
KEEPR = 2                 # chunk groups kept resident across both layers
ALPHA = 0.2

BF = mybir.dt.bfloat16
F8 = mybir.dt.float8e4
F32 = mybir.dt.float32
AX = mybir.AxisListType.X
AF = mybir.ActivationFunctionType
OP = mybir.AluOpType


def build_kernel_body(nc, tc, ctx, t_in, out_dram):
    xtp, at, w1, w2, b1c, b2c, atil, identb, wlin = (
        t_in["xtp"], t_in["at"], t_in["w1"], t_in["w2"],
        t_in["b1c"], t_in["b2c"], t_in["atil"], t_in["identb"], t_in["wlin"])

    const = ctx.enter_context(tc.tile_pool(name="const", bufs=1))
    sbuf = ctx.enter_context(tc.tile_pool(name="sbuf", bufs=2))
    atp = ctx.enter_context(tc.tile_pool(name="atp", bufs=9))
    pacc = ctx.enter_context(tc.tile_pool(name="pacc", bufs=2, space="PSUM"))
    psm = ctx.enter_context(tc.tile_pool(name="psm", bufs=2, space="PSUM"))
    dram = ctx.enter_context(tc.tile_pool(name="dram", bufs=1, space="DRAM"))

    # ---- A-chunk loads on nc.sync: keep/stream issue interleaved so the
    # DMA engines run flat out from t=0 while the PE works the early keeps.
    keep = {}
    l1_stream = {}

    def issue_chunk(r, store):
        for m in range(M):
            if store is keep:
                t = const.tile([128, L * UL], F8, name=f"keep_{m}_{r}")
            else:
                t = atp.tile([128, L * UL], F8, tag="at", name=f"at{m}")
            nc.sync.dma_start(t[:], at[m, r, :, :])
            store[(m, r)] = t

    for r in range(R):
        issue_chunk(r, keep if r < KEEPR else l1_stream)

    def chunk_tiles(r, stream):
        if r < KEEPR:
            return [keep[(m, r)] for m in range(M)]
        return [stream[(m, r)] for m in range(M)]

    # ---- constants / parameters (nc.scalar ring) ----
    xtp_sb = const.tile([128, N], BF)
    nc.scalar.dma_start(xtp_sb[:], xtp[:])
    w1_sb = const.tile([128, M * H], BF)
    nc.scalar.dma_start(w1_sb[:], w1[:])
    w2_sb = const.tile([M * H, H], BF)
    nc.scalar.dma_start(w2_sb[:], w2[:])
    b1c_sb = const.tile([M * H, 1], F32)
    nc.scalar.dma_start(b1c_sb[:], b1c[:])
    b2c_sb = const.tile([M * H, 1], F32)
    nc.scalar.dma_start(b2c_sb[:], b2c[:])
    atil_sb = const.tile([128, M * H * 8], F32)
    nc.scalar.dma_start(atil_sb[:], atil[:])
    identb_sb = const.tile([M * H, H], F32)
    nc.scalar.dma_start(identb_sb[:], identb[:])
    wlin_sb = const.tile([H + 1, C], F32)
    nc.scalar.dma_start(wlin_sb[:], wlin[:])
    ident_sb = const.tile([128, 128], F32)
    make_identity(nc, ident_sb[:])

    s1_sb = const.tile([128, R * L * M * H], BF)      # S1[v-tile t, (m,h)]
    s2r_sb = const.tile([128, R * M * L * H], BF)     # gathered S2, tile (r,m)
    h1t_sb = const.tile([M * H, UL], BF)              # rows 32m+h
    h2t_sb = const.tile([M * H, UL], F32)

    # ---- S1 = x @ (W1/N), v-tile at a time ----
    for t in range(R * L):
        ps1 = psm.tile([128, M * H], F32, tag="w96", name="ps1")
        nc.tensor.matmul(ps1[:], xtp_sb[:, t * 128:(t + 1) * 128], w1_sb[:],
                         start=True, stop=True)
        nc.vector.tensor_copy(s1_sb[:, t * M * H:(t + 1) * M * H], ps1[:])

    # ---- GCN layer: three metapaths concurrent in PE column groups ----
    def gcn_layer(stat, bc_sb, ht_out, stream):
        accs = [pacc.tile([128, 512], F32, tag="acc", name=f"acc{s}")
                for s in range(2)]
        for r in range(R):
            ats = chunk_tiles(r, stream)
            for l in range(L):
                t = L * r + l
                for s in range(2):
                    for m in range(M):
                        nc.tensor.matmul(
                            accs[s][32 * m:32 * m + 32, :], stat(t, m),
                            ats[m][:, l * UL + s * 512: l * UL + s * 512 + 512],
                            start=(t == 0), stop=(t == R * L - 1),
                            skip_group_check=True)
        for s in range(2):
            for m in range(M):
                nc.scalar.activation(
                    ht_out[32 * m:32 * m + 32, s * 512:(s + 1) * 512],
                    accs[s][32 * m:32 * m + 32, :], AF.Relu,
                    bias=bc_sb[32 * m:32 * m + 32, :])

    gcn_layer(lambda t, m: s1_sb[:, t * M * H + m * H: t * M * H + (m + 1) * H],
              b1c_sb, h1t_sb, l1_stream)

    # layer-2 re-streams of the non-kept chunks: issued on the sync ring right
    # after layer 1's, so they fill the pool while the AllGather runs.
    l2_stream = {}
    for r in range(KEEPR, R):
        issue_chunk(r, l2_stream)

    # ---- S2 = h1 @ (W2/N), copied straight into the slot layout ----
    # slot column group l of metapath block m holds rows u = 128 l + p, which
    # is exactly the ps2 tile of u-tile l.
    s2send = const.tile([128, M * L * H], BF)
    for ut in range(L):
        for m in range(M):
            ps2 = psm.tile([128, H], F32, tag="w32", name="ps2")
            nc.tensor.matmul(ps2[:], h1t_sb[32 * m:32 * m + 32, ut * 128:(ut + 1) * 128],
                             w2_sb[32 * m:32 * m + 32, :], start=True, stop=True)
            nc.vector.tensor_copy(
                s2send[:, m * L * H + ut * H:m * L * H + (ut + 1) * H], ps2[:])

    s2loc = dram.tile([128, M * L * H], BF)
    nc.scalar.dma_start(s2loc[:, :], s2send[:])
    s2full = dram.tile([NCORES, 128, M * L * H], BF, addr_space="Shared")
    nc.gpsimd.collective_compute(
        "AllGather", OP.bypass,
        replica_groups=[list(range(NCORES))],
        ins=[s2loc[:].opt()], outs=[s2full[:].opt()])
    nc.scalar.dma_start(s2r_sb[:].rearrange("p (r c) -> p r c", r=NCORES),
                        s2full[:, :, :].rearrange("r p c -> p r c"))

    gcn_layer(lambda t, m: s2r_sb[:, (t // L) * M * L * H + m * L * H + (t % L) * H:
                                  (t // L) * M * L * H + m * L * H + (t % L + 1) * H],
              b2c_sb, h2t_sb, l2_stream)

    # ---- tail: metapath attention + linear head, batched over u-tiles ----
    h2u = const.tile([128, L * M * H], F32)       # [u, (ut, m, h)]
    for ut in range(L):
        for m in range(M):
            trp = psm.tile([128, H], F32, tag="w32", name="trp")
            nc.tensor.transpose(trp[:], h2t_sb[32 * m:32 * m + 32,
                                               ut * 128:(ut + 1) * 128],
                                identb_sb[32 * m:32 * m + 32, :])
            if (ut * M + m) % 2 == 0:
                nc.vector.tensor_copy(
                    h2u[:, (ut * M + m) * H:(ut * M + m + 1) * H], trp[:])
            else:
                nc.scalar.copy(
                    h2u[:, (ut * M + m) * H:(ut * M + m + 1) * H], trp[:])

    tmp = sbuf.tile([128, L * M * H], F32, tag="tmp", name="tmp")
    nc.vector.tensor_mul(tmp[:], h2u[:], atil_sb[:])
    e_all = sbuf.tile([128, L * M], F32, tag="eall", name="e_all")
    nc.vector.reduce_sum(e_all[:].rearrange("p (a o) -> p a o", o=1),
                         tmp[:].rearrange("p (a h) -> p a h", h=H), axis=AX)
    # leaky relu then unshifted softmax over m (e is tiny; exp is safe)
    eta = sbuf.tile([128, L * M], F32, tag="eta", name="eta")
    nc.vector.tensor_scalar_mul(eta[:], e_all[:], ALPHA)
    el = sbuf.tile([128, L * M], F32, tag="el", name="el")
    nc.vector.tensor_max(el[:], e_all[:], eta[:])
    ex = sbuf.tile([128, L * M], F32, tag="ex", name="ex")
    nc.scalar.activation(ex[:], el[:], AF.Exp)
    ssum = sbuf.tile([128, L], F32, tag="ssum", name="ssum")
    nc.vector.reduce_sum(ssum[:].rearrange("p (a o) -> p a o", o=1),
                         ex[:].rearrange("p (a m) -> p a m", m=M), axis=AX)
    rs = sbuf.tile([128, L], F32, tag="rs", name="rs")
    nc.vector.reciprocal(rs[:], ssum[:])
    rs3 = sbuf.tile([128, L * M], F32, tag="rs3", name="rs3")
    for m in range(M):
        nc.vector.tensor_copy(
            rs3[:].rearrange("p (a m) -> p a m", m=M)[:, :, m:m + 1],
            rs[:].rearrange("p (a o) -> p a o", o=1))
    attn = sbuf.tile([128, L * M], F32, tag="attn", name="attn")
    nc.vector.tensor_mul(attn[:], ex[:], rs3[:])
    # out = sum_m attn[:, (ut,m)] * h2u[:, (ut,m,:)] — per-ut muls (the scale
    # is per (partition, ut)), vector/scalar split, then two wide adds.
    oacc = sbuf.tile([128, L * H], F32, tag="oacc", name="oacc")
    t1a = sbuf.tile([128, L * H], F32, tag="t1a", name="t1a")
    t2a = sbuf.tile([128, L * H], F32, tag="t2a", name="t2a")
    for ut in range(L):
        nc.vector.tensor_scalar_mul(oacc[:, ut * H:(ut + 1) * H],
                                    h2u[:, (ut * M) * H:(ut * M + 1) * H],
                                    attn[:, ut * M:ut * M + 1])
        nc.scalar.activation(t1a[:, ut * H:(ut + 1) * H],
                             h2u[:, (ut * M + 1) * H:(ut * M + 2) * H],
                             AF.Copy, scale=attn[:, ut * M + 1:ut * M + 2])
        nc.vector.tensor_scalar_mul(t2a[:, ut * H:(ut + 1) * H],
                                    h2u[:, (ut * M + 2) * H:(ut * M + 3) * H],
                                    attn[:, ut * M + 2:ut * M + 3])
    osum = sbuf.tile([128, L * H], F32, tag="osum", name="osum")
    nc.vector.tensor_add(osum[:], t1a[:], t2a[:])
    oaccf = sbuf.tile([128, L * H], F32, tag="oaccf", name="oaccf")
    nc.vector.tensor_add(oaccf[:], oacc[:], osum[:])
    # logits = relu([out, 1] @ [W_lin; b_lin]); log_softmax (unshifted, small)
    ota = const.tile([H + 1, L * 128], F32)
    nc.vector.memset(ota[H:H + 1, :], 1.0)
    for ut in range(L):
        otp = psm.tile([H, 128], F32, tag="w32t", name="otp")
        nc.tensor.transpose(otp[:], oaccf[:, ut * H:(ut + 1) * H], ident_sb[:])
        nc.vector.tensor_copy(ota[0:H, ut * 128:(ut + 1) * 128], otp[:])
    lgr = sbuf.tile([128, L * C], F32, tag="lgr", name="lgr")
    for ut in range(L):
        lg = psm.tile([128, C], F32, tag="w32", name="lg")
        nc.tensor.matmul(lg[:], ota[:, ut * 128:(ut + 1) * 128], wlin_sb[:],
                         start=True, stop=True)
        nc.vector.tensor_copy(lgr[:, ut * C:(ut + 1) * C], lg[:])
    lgR = sbuf.tile([128, L * C], F32, tag="lgR", name="lgR")
    nc.vector.tensor_scalar_max(lgR[:], lgr[:], 0.0)
    ex2 = sbuf.tile([128, L * C], F32, tag="ex2", name="ex2")
    nc.scalar.activation(ex2[:], lgR[:], AF.Exp)
    sm = sbuf.tile([128, L], F32, tag="sm", name="sm")
    nc.vector.reduce_sum(sm[:].rearrange("p (a o) -> p a o", o=1),
                         ex2[:].rearrange("p (a c) -> p a c", c=C), axis=AX)
    lssum = sbuf.tile([128, L], F32, tag="lssum", name="lssum")
    nc.scalar.activation(lssum[:], sm[:], AF.Ln)
    fin = sbuf.tile([128, L * C], F32, tag="fin", name="fin")
    for ut in range(L):
        nc.vector.tensor_scalar_sub(fin[:, ut * C:(ut + 1) * C],
                                    lgR[:, ut * C:(ut + 1) * C],
                                    lssum[:, ut:ut + 1])
        nc.scalar.dma_start(out_dram[ut * 128:(ut + 1) * 128, :],
                            fin[:, ut * C:(ut + 1) * C])


_CACHED = {}


def build():
    if "nc" in _CACHED:
        return _CACHED["nc"]
    nc = bacc.Bacc("TRN2", target_bir_lowering=False, debug=False,
                   num_devices=NCORES)
    t_in = {
        "xtp": nc.dram_tensor("xtp", [128, N], BF, kind="ExternalInput").ap(),
        "at": nc.dram_tensor("at", [M, R, 128, L * UL], F8,
                             kind="ExternalInput").ap(),
        "w1": nc.dram_tensor("w1", [128, M * H], BF, kind="ExternalInput").ap(),
        "w2": nc.dram_tensor("w2", [M * H, H], BF, kind="ExternalInput").ap(),
        "b1c": nc.dram_tensor("b1c", [M * H, 1], F32, kind="ExternalInput").ap(),
        "b2c": nc.dram_tensor("b2c", [M * H, 1], F32, kind="ExternalInput").ap(),
        "atil": nc.dram_tensor("atil", [128, M * H * 8], F32,
                               kind="ExternalInput").ap(),
        "identb": nc.dram_tensor("identb", [M * H, H], F32,
                                 kind="ExternalInput").ap(),
        "wlin": nc.dram_tensor("wlin", [H + 1, C], F32,
                               kind="ExternalInput").ap(),
    }
    out_dram = nc.dram_tensor("out", [UL, C], F32, kind="ExternalOutput").ap()
    with tile.TileContext(nc) as tc, ExitStack() as ctx:
        build_kernel_body(nc, tc, ctx, t_in, out_dram)
    nc.compile()
    _CACHED["nc"] = nc
    return nc


def _bf16(x):
    """Fast f32 -> bf16 with round-to-nearest-even via integer ops."""
    x = np.ascontiguousarray(x, dtype=np.float32)
    u = x.view(np.uint32)
    r = ((u + 0x7FFF + ((u >> 16) & 1)) >> 16).astype(np.uint16)
    return r.view(ml_dtypes.bfloat16)


def make_in_maps(x, adjs, W1, b1, W2, b2, a, W_lin, b_lin):
    xtp = np.ascontiguousarray(_bf16(x).T)                       # [128, N]
    w1 = np.ascontiguousarray(
        _bf16(np.asarray(W1) * (1.0 / N)).transpose(1, 0, 2)).reshape(128, M * H)
    w2 = np.ascontiguousarray(
        _bf16(np.asarray(W2) * (1.0 / N))).reshape(M * H, H)
    b1c = np.ascontiguousarray(b1, dtype=np.float32).reshape(M * H, 1)
    b2c = np.ascontiguousarray(b2, dtype=np.float32).reshape(M * H, 1)
    atil = np.tile(np.asarray(a, dtype=np.float32), (128, L * M))
    identb = np.tile(np.eye(H, dtype=np.float32), (M, 1))
    wlin = np.concatenate([W_lin, b_lin[None, :]], axis=0).astype(np.float32)
    adjs8 = (np.asarray(adjs, dtype=np.float32) * float(N)).astype(
        ml_dtypes.float8_e4m3)                                   # [M, N, N]
    in_maps = []
    for k in range(NCORES):
        # chunk r: [128 p, (l, u)] with v = 1024 r + 128 l + p
        atk = np.ascontiguousarray(
            adjs8[:, k * UL:(k + 1) * UL, :].transpose(0, 2, 1)
            .reshape(M, R, L, 128, UL).transpose(0, 1, 3, 2, 4)
        ).reshape(M, R, 128, L * UL)
        in_maps.append({"xtp": xtp, "at": atk, "w1": w1, "w2": w2,
                        "b1c": b1c, "b2c": b2c, "atil": atil,
                        "identb": identb, "wlin": wlin})
    return in_maps


def kernel(x, adjs, W1, b1, W2, b2, a, W_lin, b_lin, _trace=False):
    nc = build()
    in_maps = make_in_maps(x, adjs, W1, b1, W2, b2, a, W_lin, b_lin)
    res = run_bass_kernel_spmd(nc, in_maps, core_ids=list(range(NCORES)),
                               trace=_trace)
    out = np.concatenate([res.results[k]["out"] for k in range(NCORES)], axis=0)
    if _trace:
        kernel.last_result = res
    return out
